# revision 4
# baseline (speedup 1.0000x reference)
"""Trainium2 Bass kernel for ContextShadowReasoning (dense CNN + per-image attention).

Strategy (8 NeuronCores): data-parallel over B=4 images x 2-way spatial split
(rows 0-31 / 32-63) per image.  Each core computes its half of
multi = conv7x7(x) (+) dilated-conv5x5(x) with w_comb folded in host-side,
then q/k/vT for its half; a pairwise AllGather exchanges (k, vT) so each core
holds the full-image k/v; attention output for the core's 2048 query pixels is
computed with scores transposed (n on partitions) so the softmax reduction
folds into the AV matmul via an appended ones-column in vT.

k/vT are stored in [own-half | peer-half] local order: the own half is written
directly by the q/k/v stage (no collective dependency), the peer half is
fetched from the AllGather result with indirect DMAs driven by per-core index
inputs.  Attention n-blocks 0-15 therefore overlap with the collective; only
blocks 16-31 wait on it.
"""

import sys

for _p in ("/opt/trn_rl_repo", "/root/.axon_site/_ro/trn_rl_repo"):
    if _p not in sys.path:
        sys.path.insert(0, _p)

import numpy as np
import ml_dtypes

import concourse.bass as bass
import concourse.mybir as mybir
import concourse.tile as tile
from concourse import bacc
from concourse.bass_utils import run_bass_kernel_spmd

F32 = mybir.dt.float32
BF16 = mybir.dt.bfloat16
I32 = mybir.dt.int32
AF = mybir.ActivationFunctionType
bf16 = ml_dtypes.bfloat16

B, C, H, W = 4, 256, 64, 64
C8 = C // 8          # 32
N = H * W            # 4096
NCORES = 8
HH = H // 2          # 32 rows per half
M = HH * W           # 2048 queries per core
PAD = 6              # max halo (dilated 5x5, dil=3)
HP, WP = HH + 2 * PAD, W + 2 * PAD    # 44, 76
XPLEN = HP * WP                        # 3344
NTAPS = 74           # 49 (7x7 dil 1) + 25 (5x5 dil 3)
NB_OWN = M // 128    # 16 n-blocks per half
NB_ALL = N // 128    # 32 n-blocks full image
VTW = C + 1          # 257: v channels + ones column

# tap t -> (dh, dw) offsets, matching host weight packing order
TAPS = [(kh - 3, kw - 3) for kh in range(7) for kw in range(7)] + [
    (3 * a - 6, 3 * b - 6) for a in range(5) for b in range(5)
]
assert len(TAPS) == NTAPS

_PROGRAM = None


def _build_program():
    global _PROGRAM
    if _PROGRAM is not None:
        return _PROGRAM

    nc = bacc.Bacc("TRN2", target_bir_lowering=False, debug=False,
                   num_devices=NCORES)

    XPAD = nc.dram_tensor("XPAD", [2, 128, XPLEN], BF16, kind="ExternalInput").ap()
    WCONV = nc.dram_tensor("WCONV", [2, 2, 128, NTAPS * 128], BF16, kind="ExternalInput").ap()
    BEFF = nc.dram_tensor("BEFF", [128, 2], F32, kind="ExternalInput").ap()
    WQ = nc.dram_tensor("WQ", [2, 128, C8], BF16, kind="ExternalInput").ap()
    BQ = nc.dram_tensor("BQ", [C8, 1], F32, kind="ExternalInput").ap()
    WK = nc.dram_tensor("WK", [2, 128, C8], BF16, kind="ExternalInput").ap()
    BK = nc.dram_tensor("BK", [C8, 1], F32, kind="ExternalInput").ap()
    WVA = nc.dram_tensor("WVA", [2, 128, VTW], BF16, kind="ExternalInput").ap()
    BVA = nc.dram_tensor("BVA", [1, VTW], BF16, kind="ExternalInput").ap()
    MASKF = nc.dram_tensor("MASKF", [16, 128], F32, kind="ExternalInput").ap()
    KIDX = nc.dram_tensor("KIDX", [C8], I32, kind="ExternalInput").ap()
    PIDX = nc.dram_tensor("PIDX", [16, 128], I32, kind="ExternalInput").ap()
    OUT = nc.dram_tensor("OUT", [M, C], F32, kind="ExternalOutput").ap()

    with tile.TileContext(nc) as tc:
        with (
            tc.tile_pool(name="persist", bufs=1) as pp,
            tc.tile_pool(name="dram", bufs=1, space="DRAM") as dramp,
        ):
            # ---- persistent SBUF state ----
            xpad = [pp.tile([128, XPLEN], BF16, tag=f"xpad{cb}", name=f"xpad{cb}") for cb in range(2)]
            for cb in range(2):
                nc.sync.dma_start(xpad[cb], XPAD[cb])
            xp3 = [t[:].rearrange("p (h w) -> p h w", h=HP) for t in xpad]

            multi = [pp.tile([128, M], BF16, tag=f"multi{cb}", name=f"multi{cb}") for cb in range(2)]
            beff = pp.tile([128, 2], F32); nc.sync.dma_start(beff, BEFF)
            wq = [pp.tile([128, C8], BF16, tag=f"wq{cb}", name=f"wq{cb}") for cb in range(2)]
            wk = [pp.tile([128, C8], BF16, tag=f"wk{cb}", name=f"wk{cb}") for cb in range(2)]
            wva = [pp.tile([128, VTW], BF16, tag=f"wva{cb}", name=f"wva{cb}") for cb in range(2)]
            for cb in range(2):
                nc.sync.dma_start(wq[cb], WQ[cb])
                nc.sync.dma_start(wk[cb], WK[cb])
                nc.sync.dma_start(wva[cb], WVA[cb])
            bq = pp.tile([C8, 1], F32); nc.sync.dma_start(bq, BQ)
            bk = pp.tile([C8, 1], F32); nc.sync.dma_start(bk, BK)
            bva = pp.tile([1, VTW], BF16); nc.sync.dma_start(bva, BVA)
            ones1 = pp.tile([1, 128], BF16); nc.vector.memset(ones1, 1.0)
            maskf = pp.tile([128, 16], F32)
            for mb in range(16):
                nc.sync.dma_start(maskf[:, mb:mb + 1], MASKF[mb].unsqueeze(1))
            kidx = pp.tile([C8, 1], I32); nc.sync.dma_start(kidx, KIDX.unsqueeze(1))
            pidx = pp.tile([128, 16], I32)
            for nb in range(16):
                nc.sync.dma_start(pidx[:, nb:nb + 1], PIDX[nb].unsqueeze(1))

            # local-order [own | peer] k and vT
            q_sb = pp.tile([C8, M], BF16)
            k4 = pp.tile([C8, N], BF16)
            vt = pp.tile([128, NB_ALL * VTW], BF16)

            kp_own = dramp.tile([C8, M], BF16, name="kp_own")
            kp_all = dramp.tile([2, C8, M], BF16, name="kp_all")
            vp_own = dramp.tile([M, VTW], BF16, name="vp_own")
            vp_all = dramp.tile([2 * M, VTW], BF16, name="vp_all")

            # ---- phase A: multi = folded double conv ----
            with (
                tc.tile_pool(name="wconv", bufs=2) as wpool,
                tc.tile_pool(name="psA", bufs=8, space="PSUM") as psA,
            ):
                for ob in range(2):
                    cps = [psA.tile([128, 512], F32, tag="cps", name="cps") for _ in range(4)]
                    for cb in range(2):
                        wt = wpool.tile([128, NTAPS * 128], BF16, tag="wt", name="wt")
                        nc.sync.dma_start(wt, WCONV[cb, ob])
                        for t, (dh, dw) in enumerate(TAPS):
                            lhsT = wt[:, t * 128:(t + 1) * 128]
                            for ch in range(4):
                                r0 = ch * 8 + PAD + dh
                                c0 = PAD + dw
                                nc.tensor.matmul(
                                    cps[ch], lhsT, xp3[cb][:, r0:r0 + 8, c0:c0 + W],
                                    start=(cb == 0 and t == 0),
                                    stop=(cb == 1 and t == NTAPS - 1),
                                )
                    for ch in range(4):
                        nc.scalar.activation(
                            multi[ob][:, ch * 512:(ch + 1) * 512], cps[ch],
                            AF.Identity, bias=beff[:, ob:ob + 1], scale=1.0,
                        )

            # ---- phase B: q, k, vT of own half; AllGather k/vT ----
            with (
                tc.tile_pool(name="psB", bufs=2, space="PSUM") as psB,
            ):
                for ch in range(4):
                    sl = slice(ch * 512, (ch + 1) * 512)
                    pq = psB.tile([C8, 512], F32, tag="pq", name="pq")
                    nc.tensor.matmul(pq, wq[0], multi[0][:, sl], start=True, stop=False)
                    nc.tensor.matmul(pq, wq[1], multi[1][:, sl], start=False, stop=True)
                    nc.scalar.activation(q_sb[:, sl], pq, AF.Identity,
                                         bias=bq[:, 0:1], scale=1.0)
                    pk = psB.tile([C8, 512], F32, tag="pk", name="pk")
                    nc.tensor.matmul(pk, wk[0], multi[0][:, sl], start=True, stop=False)
                    nc.tensor.matmul(pk, wk[1], multi[1][:, sl], start=False, stop=True)
                    # own half lands directly in k4 columns [0:M]
                    nc.scalar.activation(k4[:, sl], pk, AF.Identity,
                                         bias=bk[:, 0:1], scale=1.0)
                    nc.sync.dma_start(kp_own[:, sl], k4[:, sl])

                for nb in range(NB_OWN):
                    pv = psB.tile([128, VTW], F32, tag="pv", name="pv")
                    lv = slice(nb * 128, (nb + 1) * 128)
                    nc.tensor.matmul(pv, multi[0][:, lv], wva[0], start=True, stop=False)
                    nc.tensor.matmul(pv, multi[1][:, lv], wva[1], start=False, stop=False)
                    nc.tensor.matmul(pv, ones1, bva, start=False, stop=True)
                    vsl = slice(nb * VTW, (nb + 1) * VTW)
                    nc.scalar.activation(vt[:, vsl], pv, AF.Identity, scale=1.0)
                    nc.sync.dma_start(vp_own[nb * 128:(nb + 1) * 128, :], vt[:, vsl])

                nc.gpsimd.collective_compute(
                    "AllGather", mybir.AluOpType.bypass,
                    replica_groups=[[0, 1], [2, 3], [4, 5], [6, 7]],
                    ins=[kp_own[:]], outs=[kp_all[:]],
                )
                nc.gpsimd.collective_compute(
                    "AllGather", mybir.AluOpType.bypass,
                    replica_groups=[[0, 1], [2, 3], [4, 5], [6, 7]],
                    ins=[vp_own[:]], outs=[vp_all[:]],
                )
                # peer halves via indirect row gather (per-core indices)
                nc.gpsimd.indirect_dma_start(
                    out=k4[:, M:2 * M], out_offset=None,
                    in_=kp_all[:].rearrange("h p m -> (h p) m"),
                    in_offset=bass.IndirectOffsetOnAxis(ap=kidx[:, 0:1], axis=0),
                )
                for nb in range(NB_OWN):
                    g = NB_OWN + nb
                    nc.gpsimd.indirect_dma_start(
                        out=vt[:, g * VTW:(g + 1) * VTW], out_offset=None,
                        in_=vp_all[:],
                        in_offset=bass.IndirectOffsetOnAxis(ap=pidx[:, nb:nb + 1], axis=0),
                    )

            # ---- phase C: attention for own 2048 queries ----
            # n-blocks 0-15 (own) need no collective; 16-31 wait on the gather.
            with (
                tc.tile_pool(name="psAV", bufs=4, space="PSUM") as psav,
                tc.tile_pool(name="psS", bufs=2, space="PSUM") as pss,
                tc.tile_pool(name="expp", bufs=3) as expp,
                tc.tile_pool(name="outp", bufs=4) as outp,
            ):
                for mc in range(4):
                    msl = slice(mc * 512, (mc + 1) * 512)
                    av = [psav.tile([128, VTW], F32, tag="av", name="av") for _ in range(4)]
                    for nbp in range(NB_ALL // 2):
                        sc2 = pss.tile([128, 1024], F32, tag="sc2", name="sc2")
                        ex2 = expp.tile([128, 1024], BF16, tag="ex2", name="ex2")
                        for j in range(2):
                            nb = 2 * nbp + j
                            nc.tensor.matmul(sc2[:, j * 512:(j + 1) * 512],
                                             k4[:, nb * 128:(nb + 1) * 128],
                                             q_sb[:, msl], start=True, stop=True)
                        nc.scalar.activation(ex2, sc2, AF.Exp, scale=1.0)
                        for j in range(2):
                            nb = 2 * nbp + j
                            for ms in range(4):
                                nc.tensor.matmul(
                                    av[ms], ex2[:, j * 512 + ms * 128:j * 512 + (ms + 1) * 128],
                                    vt[:, nb * VTW:(nb + 1) * VTW],
                                    start=(nb == 0), stop=(nb == NB_ALL - 1),
                                )
                    for ms in range(4):
                        mb = mc * 4 + ms
                        inv = outp.tile([128, 1], F32, tag="inv", name="inv")
                        nc.vector.reciprocal(inv, av[ms][:, C:C + 1])
                        fac = outp.tile([128, 1], F32, tag="fac", name="fac")
                        nc.vector.tensor_mul(fac, inv, maskf[:, mb:mb + 1])
                        ot = outp.tile([128, C], F32, tag="ot", name="ot")
                        nc.vector.tensor_scalar_mul(ot, av[ms][:, 0:C], fac[:, 0:1])
                        nc.sync.dma_start(OUT[mb * 128:(mb + 1) * 128, :], ot)

    nc.compile()
    _PROGRAM = nc
    return nc


def _prep_in_maps(x, mask, w_ctx, b_ctx, w_wide, b_wide, w_comb, b_comb,
                  w_q, b_q, w_k, b_k, w_v, b_v):
    f8 = np.float64
    wc = w_comb[:, :, 0, 0].astype(f8)
    wl = (wc[:, :C] @ w_ctx.astype(f8).reshape(C, -1)).reshape(C, C, 7, 7)
    ww = (wc[:, C:] @ w_wide.astype(f8).reshape(C, -1)).reshape(C, C, 5, 5)
    b_eff = (wc[:, :C] @ b_ctx.astype(f8) + wc[:, C:] @ b_wide.astype(f8)
             + b_comb.astype(f8)).astype(np.float32)

    arr = np.concatenate([
        wl.transpose(1, 0, 2, 3).reshape(C, C, 49),
        ww.transpose(1, 0, 2, 3).reshape(C, C, 25),
    ], axis=2)  # [ci, co, tap]
    wconv = np.empty((2, 2, 128, NTAPS * 128), dtype=bf16)
    for cb in range(2):
        for ob in range(2):
            blk = arr[cb * 128:(cb + 1) * 128, ob * 128:(ob + 1) * 128, :]
            wconv[cb, ob] = blk.transpose(0, 2, 1).reshape(128, NTAPS * 128).astype(bf16)

    beff_np = np.stack([b_eff[:128], b_eff[128:]], axis=1).astype(np.float32)

    wq_np = np.stack([w_q[:, :128, 0, 0].T, w_q[:, 128:, 0, 0].T]).astype(bf16)
    wk_np = np.stack([w_k[:, :128, 0, 0].T, w_k[:, 128:, 0, 0].T]).astype(bf16)
    wva_np = np.zeros((2, 128, VTW), dtype=bf16)
    wva_np[0, :, :C] = w_v[:, :128, 0, 0].T.astype(bf16)
    wva_np[1, :, :C] = w_v[:, 128:, 0, 0].T.astype(bf16)
    bva_np = np.zeros((1, VTW), dtype=bf16)
    bva_np[0, :C] = b_v.astype(bf16)
    bva_np[0, C] = 1.0
    bq_np = b_q.astype(np.float32)[:, None]
    bk_np = b_k.astype(np.float32)[:, None]

    shared = {
        "WCONV": wconv, "BEFF": beff_np,
        "WQ": wq_np, "BQ": bq_np, "WK": wk_np, "BK": bk_np,
        "WVA": wva_np, "BVA": bva_np,
    }

    in_maps = []
    x32 = x.astype(np.float32)
    mask_r = mask[:, 0, ::4, ::4].astype(np.float32)  # nearest resize 256->64
    for core in range(NCORES):
        b, s = core // 2, core % 2
        xpad = np.zeros((2, 128, HP, WP), dtype=np.float32)
        g0 = s * HH - PAD
        lo, hi = max(0, -g0), min(HP, H - g0)
        for cb in range(2):
            xpad[cb, :, lo:hi, PAD:PAD + W] = x32[b, cb * 128:(cb + 1) * 128,
                                                  g0 + lo:g0 + hi, :]
        maskf = (1.0 + mask_r[b, s * HH:(s + 1) * HH, :]).reshape(16, 128)
        peer = 1 - s
        kidx = (peer * C8 + np.arange(C8)).astype(np.int32)
        pidx = (peer * M + np.arange(M)).astype(np.int32).reshape(16, 128)
        in_maps.append({
            "XPAD": xpad.reshape(2, 128, XPLEN).astype(bf16),
            "MASKF": maskf.astype(np.float32),
            "KIDX": kidx, "PIDX": pidx,
            **shared,
        })
    return in_maps


def _assemble(results):
    out = np.empty((B, C, H, W), dtype=np.float32)
    for core in range(NCORES):
        b, s = core // 2, core % 2
        out[b, :, s * HH:(s + 1) * HH, :] = \
            results[core]["OUT"].T.reshape(C, HH, W)
    return out


def kernel(**inputs):
    nc = _build_program()
    in_maps = _prep_in_maps(**inputs)
    res = run_bass_kernel_spmd(nc, in_maps, core_ids=list(range(NCORES)))
    return _assemble(res.results)


# revision 5
# speedup vs baseline: 196.6565x; 196.6565x over previous
"""Trainium2 Bass kernel for ContextShadowReasoning (dense CNN + per-image attention).

Strategy (8 NeuronCores): data-parallel over B=4 images x 2-way spatial split
(rows 0-31 / 32-63) per image.  Each core computes its half of
multi = conv7x7(x) (+) dilated-conv5x5(x) with w_comb folded in host-side,
then q/k/vT for its half; a pairwise AllGather exchanges (k, vT) so each core
holds the full-image k/v; attention output for the core's 2048 query pixels is
computed with scores transposed (n on partitions) so softmax reduction folds
into the AV matmul via an appended ones-column in vT.
"""

import sys

for _p in ("/opt/trn_rl_repo", "/root/.axon_site/_ro/trn_rl_repo"):
    if _p not in sys.path:
        sys.path.insert(0, _p)

import numpy as np
import ml_dtypes

import concourse.mybir as mybir
import concourse.tile as tile
from concourse import bacc
from concourse.bass_utils import run_bass_kernel_spmd

F32 = mybir.dt.float32
BF16 = mybir.dt.bfloat16
AF = mybir.ActivationFunctionType
bf16 = ml_dtypes.bfloat16

B, C, H, W = 4, 256, 64, 64
C8 = C // 8          # 32
N = H * W            # 4096
NCORES = 8
HH = H // 2          # 32 rows per half
M = HH * W           # 2048 queries per core
PAD = 6              # max halo (dilated 5x5, dil=3)
HP, WP = HH + 2 * PAD, W + 2 * PAD    # 44, 76
XPLEN = HP * WP                        # 3344
NTAPS = 74           # 49 (7x7 dil 1) + 25 (5x5 dil 3)
NB_OWN = M // 128    # 16 n-blocks per half
NB_ALL = N // 128    # 32 n-blocks full image
VTW = C + 1          # 257: v channels + ones column
KV_K = C8 * M                 # 65536 elems of k half
KV_V = M * VTW                # 526336 elems of vT half
KV_LEN = KV_K + KV_V          # 591872

# tap t -> (dh, dw) offsets, matching host weight packing order
TAPS = [(kh - 3, kw - 3) for kh in range(7) for kw in range(7)] + [
    (3 * a - 6, 3 * b - 6) for a in range(5) for b in range(5)
]
assert len(TAPS) == NTAPS

_PROGRAM = None  # cached (nc,) build


def _build_program():
    global _PROGRAM
    if _PROGRAM is not None:
        return _PROGRAM

    nc = bacc.Bacc("TRN2", target_bir_lowering=False, debug=False,
                   num_devices=NCORES)

    XPAD = nc.dram_tensor("XPAD", [2, 128, XPLEN], BF16, kind="ExternalInput").ap()
    WCONV = nc.dram_tensor("WCONV", [2, 2, 128, NTAPS * 128], BF16, kind="ExternalInput").ap()
    BEFF = nc.dram_tensor("BEFF", [128, 2], F32, kind="ExternalInput").ap()
    WQ = nc.dram_tensor("WQ", [2, 128, C8], BF16, kind="ExternalInput").ap()
    BQ = nc.dram_tensor("BQ", [C8, 1], F32, kind="ExternalInput").ap()
    WK = nc.dram_tensor("WK", [2, 128, C8], BF16, kind="ExternalInput").ap()
    BK = nc.dram_tensor("BK", [C8, 1], F32, kind="ExternalInput").ap()
    WVA = nc.dram_tensor("WVA", [2, 128, VTW], BF16, kind="ExternalInput").ap()
    BVA = nc.dram_tensor("BVA", [1, VTW], BF16, kind="ExternalInput").ap()
    MASKF = nc.dram_tensor("MASKF", [16, 128], F32, kind="ExternalInput").ap()
    OUT = nc.dram_tensor("OUT", [M, C], F32, kind="ExternalOutput").ap()

    with tile.TileContext(nc) as tc:
        with (
            tc.tile_pool(name="persist", bufs=1) as pp,
            tc.tile_pool(name="dram", bufs=1, space="DRAM") as dramp,
        ):
            # ---- persistent SBUF state ----
            xpad = [pp.tile([128, XPLEN], BF16, tag=f"xpad{cb}", name=f"xpad{cb}") for cb in range(2)]
            for cb in range(2):
                nc.sync.dma_start(xpad[cb], XPAD[cb])
            xp3 = [t[:].rearrange("p (h w) -> p h w", h=HP) for t in xpad]

            multi = [pp.tile([128, M], BF16, tag=f"multi{cb}", name=f"multi{cb}") for cb in range(2)]
            beff = pp.tile([128, 2], F32); nc.sync.dma_start(beff, BEFF)
            wq = [pp.tile([128, C8], BF16, tag=f"wq{cb}", name=f"wq{cb}") for cb in range(2)]
            wk = [pp.tile([128, C8], BF16, tag=f"wk{cb}", name=f"wk{cb}") for cb in range(2)]
            wva = [pp.tile([128, VTW], BF16, tag=f"wva{cb}", name=f"wva{cb}") for cb in range(2)]
            for cb in range(2):
                nc.sync.dma_start(wq[cb], WQ[cb])
                nc.sync.dma_start(wk[cb], WK[cb])
                nc.sync.dma_start(wva[cb], WVA[cb])
            bq = pp.tile([C8, 1], F32); nc.sync.dma_start(bq, BQ)
            bk = pp.tile([C8, 1], F32); nc.sync.dma_start(bk, BK)
            bva = pp.tile([1, VTW], BF16); nc.sync.dma_start(bva, BVA)
            ones1 = pp.tile([1, 128], BF16); nc.vector.memset(ones1, 1.0)
            maskf = pp.tile([128, 16], F32)
            for mb in range(16):
                nc.sync.dma_start(maskf[:, mb:mb + 1], MASKF[mb].unsqueeze(1))

            q_sb = pp.tile([C8, M], BF16)
            k4 = pp.tile([C8, N], BF16)
            vt = pp.tile([128, NB_ALL * VTW], BF16)

            kv_own = dramp.tile([KV_LEN], BF16)
            kv_all = dramp.tile([2, KV_LEN], BF16)

            # ---- phase A: multi = folded double conv ----
            with (
                tc.tile_pool(name="wconv", bufs=2) as wpool,
                tc.tile_pool(name="psA", bufs=8, space="PSUM") as psA,
            ):
                for ob in range(2):
                    cps = [psA.tile([128, 512], F32, tag="cps", name="cps") for _ in range(4)]
                    for cb in range(2):
                        wt = wpool.tile([128, NTAPS * 128], BF16, tag="wt", name="wt")
                        nc.sync.dma_start(wt, WCONV[cb, ob])
                        for t, (dh, dw) in enumerate(TAPS):
                            lhsT = wt[:, t * 128:(t + 1) * 128]
                            for ch in range(4):
                                r0 = ch * 8 + PAD + dh
                                c0 = PAD + dw
                                nc.tensor.matmul(
                                    cps[ch], lhsT, xp3[cb][:, r0:r0 + 8, c0:c0 + W],
                                    start=(cb == 0 and t == 0),
                                    stop=(cb == 1 and t == NTAPS - 1),
                                )
                    for ch in range(4):
                        nc.scalar.activation(
                            multi[ob][:, ch * 512:(ch + 1) * 512], cps[ch],
                            AF.Identity, bias=beff[:, ob:ob + 1], scale=1.0,
                        )

            # ---- phase B: q, k, vT of own half; AllGather k/vT ----
            with (
                tc.tile_pool(name="psB", bufs=2, space="PSUM") as psB,
                tc.tile_pool(name="stage", bufs=4) as stg,
            ):
                for ch in range(4):
                    sl = slice(ch * 512, (ch + 1) * 512)
                    pq = psB.tile([C8, 512], F32, tag="pq", name="pq")
                    nc.tensor.matmul(pq, wq[0], multi[0][:, sl], start=True, stop=False)
                    nc.tensor.matmul(pq, wq[1], multi[1][:, sl], start=False, stop=True)
                    nc.scalar.activation(q_sb[:, sl], pq, AF.Identity,
                                         bias=bq[:, 0:1], scale=1.0)
                    pk = psB.tile([C8, 512], F32, tag="pk", name="pk")
                    nc.tensor.matmul(pk, wk[0], multi[0][:, sl], start=True, stop=False)
                    nc.tensor.matmul(pk, wk[1], multi[1][:, sl], start=False, stop=True)
                    kst = stg.tile([C8, 512], BF16, tag="kst", name="kst")
                    nc.scalar.activation(kst, pk, AF.Identity, bias=bk[:, 0:1], scale=1.0)
                    nc.sync.dma_start(
                        kv_own[0:KV_K].rearrange("(p n) -> p n", p=C8)[:, sl], kst)

                for nb in range(NB_OWN):
                    pv = psB.tile([128, VTW], F32, tag="pv", name="pv")
                    lv = slice(nb * 128, (nb + 1) * 128)
                    nc.tensor.matmul(pv, multi[0][:, lv], wva[0], start=True, stop=False)
                    nc.tensor.matmul(pv, multi[1][:, lv], wva[1], start=False, stop=False)
                    nc.tensor.matmul(pv, ones1, bva, start=False, stop=True)
                    vst = stg.tile([128, VTW], BF16, tag="vst", name="vst")
                    nc.scalar.activation(vst, pv, AF.Identity, scale=1.0)
                    nc.sync.dma_start(
                        kv_own[KV_K + nb * 128 * VTW: KV_K + (nb + 1) * 128 * VTW]
                        .rearrange("(p n) -> p n", p=128), vst)

                nc.gpsimd.collective_compute(
                    "AllGather", mybir.AluOpType.bypass,
                    replica_groups=[[0, 1], [2, 3], [4, 5], [6, 7]],
                    ins=[kv_own[:]], outs=[kv_all[:]],
                )
                for half in range(2):
                    nc.sync.dma_start(
                        k4[:, half * M:(half + 1) * M],
                        kv_all[half, 0:KV_K].rearrange("(p n) -> p n", p=C8))
                    for nb in range(NB_OWN):
                        g = half * NB_OWN + nb
                        nc.sync.dma_start(
                            vt[:, g * VTW:(g + 1) * VTW],
                            kv_all[half, KV_K + nb * 128 * VTW: KV_K + (nb + 1) * 128 * VTW]
                            .rearrange("(p n) -> p n", p=128))

            # ---- phase C: attention for own 2048 queries ----
            with (
                tc.tile_pool(name="psAV", bufs=4, space="PSUM") as psav,
                tc.tile_pool(name="psS", bufs=2, space="PSUM") as pss,
                tc.tile_pool(name="expp", bufs=3) as expp,
                tc.tile_pool(name="outp", bufs=4) as outp,
            ):
                for mc in range(4):
                    msl = slice(mc * 512, (mc + 1) * 512)
                    av = [psav.tile([128, VTW], F32, tag="av", name="av") for _ in range(4)]
                    for nbp in range(NB_ALL // 2):
                        sc2 = pss.tile([128, 1024], F32, tag="sc2", name="sc2")
                        ex2 = expp.tile([128, 1024], BF16, tag="ex2", name="ex2")
                        for j in range(2):
                            nb = 2 * nbp + j
                            nc.tensor.matmul(sc2[:, j * 512:(j + 1) * 512],
                                             k4[:, nb * 128:(nb + 1) * 128],
                                             q_sb[:, msl], start=True, stop=True)
                        nc.scalar.activation(ex2, sc2, AF.Exp, scale=1.0)
                        for j in range(2):
                            nb = 2 * nbp + j
                            for ms in range(4):
                                nc.tensor.matmul(
                                    av[ms], ex2[:, j * 512 + ms * 128:j * 512 + (ms + 1) * 128],
                                    vt[:, nb * VTW:(nb + 1) * VTW],
                                    start=(nb == 0), stop=(nb == NB_ALL - 1),
                                )
                    for ms in range(4):
                        mb = mc * 4 + ms
                        inv = outp.tile([128, 1], F32, tag="inv", name="inv")
                        nc.vector.reciprocal(inv, av[ms][:, C:C + 1])
                        fac = outp.tile([128, 1], F32, tag="fac", name="fac")
                        nc.vector.tensor_mul(fac, inv, maskf[:, mb:mb + 1])
                        ot = outp.tile([128, C], F32, tag="ot", name="ot")
                        nc.vector.tensor_scalar_mul(ot, av[ms][:, 0:C], fac[:, 0:1])
                        nc.sync.dma_start(OUT[mb * 128:(mb + 1) * 128, :], ot)

    nc.compile()
    _PROGRAM = nc
    return nc


def _prep_in_maps(x, mask, w_ctx, b_ctx, w_wide, b_wide, w_comb, b_comb,
                  w_q, b_q, w_k, b_k, w_v, b_v):
    f8 = np.float64
    wc = w_comb[:, :, 0, 0].astype(f8)
    wl = (wc[:, :C] @ w_ctx.astype(f8).reshape(C, -1)).reshape(C, C, 7, 7)
    ww = (wc[:, C:] @ w_wide.astype(f8).reshape(C, -1)).reshape(C, C, 5, 5)
    b_eff = (wc[:, :C] @ b_ctx.astype(f8) + wc[:, C:] @ b_wide.astype(f8)
             + b_comb.astype(f8)).astype(np.float32)

    arr = np.concatenate([
        wl.transpose(1, 0, 2, 3).reshape(C, C, 49),
        ww.transpose(1, 0, 2, 3).reshape(C, C, 25),
    ], axis=2)  # [ci, co, tap]
    wconv = np.empty((2, 2, 128, NTAPS * 128), dtype=bf16)
    for cb in range(2):
        for ob in range(2):
            blk = arr[cb * 128:(cb + 1) * 128, ob * 128:(ob + 1) * 128, :]
            wconv[cb, ob] = blk.transpose(0, 2, 1).reshape(128, NTAPS * 128).astype(bf16)

    beff_np = np.stack([b_eff[:128], b_eff[128:]], axis=1).astype(np.float32)

    wq_np = np.stack([w_q[:, :128, 0, 0].T, w_q[:, 128:, 0, 0].T]).astype(bf16)
    wk_np = np.stack([w_k[:, :128, 0, 0].T, w_k[:, 128:, 0, 0].T]).astype(bf16)
    wva_np = np.zeros((2, 128, VTW), dtype=bf16)
    wva_np[0, :, :C] = w_v[:, :128, 0, 0].T.astype(bf16)
    wva_np[1, :, :C] = w_v[:, 128:, 0, 0].T.astype(bf16)
    bva_np = np.zeros((1, VTW), dtype=bf16)
    bva_np[0, :C] = b_v.astype(bf16)
    bva_np[0, C] = 1.0
    bq_np = b_q.astype(np.float32)[:, None]
    bk_np = b_k.astype(np.float32)[:, None]

    shared = {
        "WCONV": wconv, "BEFF": beff_np,
        "WQ": wq_np, "BQ": bq_np, "WK": wk_np, "BK": bk_np,
        "WVA": wva_np, "BVA": bva_np,
    }

    in_maps = []
    x32 = x.astype(np.float32)
    mask_r = mask[:, 0, ::4, ::4].astype(np.float32)  # nearest resize 256->64
    for core in range(NCORES):
        b, s = core // 2, core % 2
        xpad = np.zeros((2, 128, HP, WP), dtype=np.float32)
        g0 = s * HH - PAD
        lo, hi = max(0, -g0), min(HP, H - g0)
        for cb in range(2):
            xpad[cb, :, lo:hi, PAD:PAD + W] = x32[b, cb * 128:(cb + 1) * 128,
                                                  g0 + lo:g0 + hi, :]
        maskf = (1.0 + mask_r[b, s * HH:(s + 1) * HH, :]).reshape(16, 128)
        in_maps.append({
            "XPAD": xpad.reshape(2, 128, XPLEN).astype(bf16),
            "MASKF": maskf.astype(np.float32),
            **shared,
        })
    return in_maps


def _assemble(results):
    out = np.empty((B, C, H, W), dtype=np.float32)
    for core in range(NCORES):
        b, s = core // 2, core % 2
        out[b, :, s * HH:(s + 1) * HH, :] = \
            results[core]["OUT"].T.reshape(C, HH, W)
    return out


def kernel(**inputs):
    nc = _build_program()
    in_maps = _prep_in_maps(**inputs)
    res = run_bass_kernel_spmd(nc, in_maps, core_ids=list(range(NCORES)))
    return _assemble(res.results)
